# revision 23
# baseline (speedup 1.0000x reference)
"""Multi-head attention (RoPE + softmax) Trainium2 kernel, 8 NeuronCores.

Sharding: B=2 batches x 16 heads -> each core owns one batch and 4 heads
(2 head-PAIRS); Wq/Wk/Wv split column-wise by head, Wo row-wise; the
per-batch partial Wo-outputs are summed on the host.

v2 dataflow (per core):
  1. QKV projection from x^T (h on partitions), kt-outer accumulation
     into 8 PSUM banks so matmuls start as soon as each x chunk lands.
     q^T,k^T [128,S] fp16 per PAIR (head a rows 0-63, head b 64-127);
     v [t][128,260] with a ones-column per head (AV emits denominators).
  2. RoPE: partition-rotation via PE perm matmul + q*cos + rot(q)*sin on
     DVE (sign folded into sin table). Layout stays pair-stacked.
  3. Attention per (pair, 1024-wide q-block): scores via ROW-TILED
     concurrent matmuls (contraction 64, head a at array rows 0-63,
     head b at 64-127 -> 2x PE throughput). Mask applied as per-key-tile
     partition bias in the exp (exact, on Act; a Schraudolph DVE path
     exists behind DVE_KT but measured slower on HW and is disabled).
  4. AV accumulation oP[65,512] fp32 (ones-col gives softmax sums in
     row 64); normalize via selector-matmul broadcast of 1/sums.
  5. Per-qb normalize + Wo (reusing attention PSUM tags between
     blocks); y staged fp16 via Act/DVE and upcast on the host, halving
     output HBM/DMA traffic.

This container's walrus accepts only ONE sync-wait per instruction; Tile
emits many multi-wait instructions, so `_split_multiwait` rewrites them
into chains of single-wait NoOps before compiling.
"""

import os
import sys

for _p in ("/opt/trn_rl_repo", "/root/.axon_site/_ro/trn_rl_repo"):
    if os.path.isdir(_p) and _p not in sys.path:
        sys.path.insert(0, _p)

import contextlib

import numpy as np

import bass_rust
import concourse.bass as bass
import concourse.tile as tile
from concourse import mybir
from concourse.bass_utils import run_bass_kernel_spmd

B, S, H = 2, 2048, 1024
NH, HD = 16, 64
ROPE_BASE = 10000.0
N_CORES = 8
HPC = 4                # heads per core
DL = HPC * HD          # local head dims per core (256)
F32 = mybir.dt.float32
F32R = mybir.dt.float32r
FP16 = mybir.dt.float16
I32 = mybir.dt.int32
AF = mybir.ActivationFunctionType
OP = mybir.AluOpType

KT = S // 128          # 16 key tiles
QT = S // 512          # 4 query tiles of 512
HT = H // 128          # 8 hidden k-tiles

MASK_NEG = -60000.0    # large-negative mask bias, finite in fp16

# Schraudolph exp: bitcast(int32(A*x + B)) ~= exp(x)
SCH_C = 366393.0
A_SCH = float(2**23 / np.log(2.0))
B_SCH = float(127 * 2**23 - SCH_C)
DVE_KT = int(os.environ.get("KRN_DVE_KT", "0"))  # head-b key tiles on DVE exp

_np_dt = {F32R: np.float32, F32: np.float32, FP16: np.float16}


def _split_multiwait(nc):
    """Split multi-wait instructions into single-wait NoOp chains."""
    n_new = 0
    for f in nc.m.functions:
        for b in f.blocks:
            il = b.instructions
            i = 0
            while i < len(il):
                ins = il[i]
                si = getattr(ins, "sync_info", None)
                if si is not None and si.on_wait is not None and len(si.on_wait) > 1:
                    waits = list(si.on_wait)
                    ups = list(si.on_update) if si.on_update else []
                    ins.sync_info = bass_rust.SyncInfo(on_wait=[waits[-1]], on_update=ups)
                    nops = []
                    for j, w in enumerate(waits[:-1]):
                        nop = bass_rust.InstNoOp(
                            name=f"{ins.name}-w{j}",
                            engine=ins.engine,
                            sync_info=bass_rust.SyncInfo(on_wait=[w], on_update=[]),
                            bass_nofuse=True,
                        )
                        nops.append(nop)
                    il[i:i] = nops
                    n_new += len(nops)
                    i += len(nops)
                i += 1
    return n_new


def _emit_body(nc, tc, d, phases=4):
    with contextlib.ExitStack() as ctx:
        const = ctx.enter_context(tc.tile_pool(name="const", bufs=1))

        wqk_sb = const.tile([128, HT, 512], FP16, name="wqk_sb")
        nc.sync.dma_start(out=wqk_sb[:],
                          in_=d["wqkT"].rearrange("(k p) m -> p k m", p=128)[:])
        x_sb = const.tile([128, HT, S], FP16, name="x_sb")
        xr = d["xT"].rearrange("(k p) m -> p k m", p=128)
        for kt in range(HT):
            nc.sync.dma_start(out=x_sb[:, kt, :], in_=xr[:, kt, :])
        wv_sb = const.tile([128, HT, 260], FP16, name="wv_sb")
        nc.sync.dma_start(out=wv_sb[:],
                          in_=d["wvT"].rearrange("(k p) m -> p k m", p=128)[:])
        cos_sb = const.tile([128, S], FP16, name="cos_sb")
        nc.sync.dma_start(out=cos_sb[:], in_=d["cosT"][:])
        sin_sb = const.tile([128, S], FP16, name="sin_sb")
        nc.sync.dma_start(out=sin_sb[:], in_=d["sinT"][:])
        perm_sb = const.tile([128, 128], FP16, name="perm_sb")
        nc.sync.dma_start(out=perm_sb[:], in_=d["perm"][:])
        mask_sb = const.tile([128, KT], F32, name="mask_sb")
        nc.sync.dma_start(out=mask_sb[:], in_=d["maskb"][:])
        schb_sb = const.tile([128, KT], F32, name="schb_sb")
        nc.sync.dma_start(out=schb_sb[:], in_=d["schb"][:])
        sel_sb = const.tile([4, 256], F32R, name="sel_sb")
        nc.sync.dma_start(out=sel_sb[:], in_=d["sel"].bitcast(F32R)[:])
        wo_sb = const.tile([128, 2, H], FP16, name="wo_sb")
        nc.sync.dma_start(out=wo_sb[:],
                          in_=d["woT"].rearrange("(k p) m -> p k m", p=128)[:])

        # persistent activations
        actp = ctx.enter_context(tc.tile_pool(name="actp", bufs=1))
        qkraw = [actp.tile([128, S], FP16, name=f"qkraw{m}") for m in range(4)]
        qrope = [actp.tile([128, S], FP16, name=f"qrope{p}") for p in range(2)]
        krope = [actp.tile([128, S], FP16, name=f"krope{p}") for p in range(2)]
        v_sb = [actp.tile([128, 260], FP16, name=f"v{t}") for t in range(KT)]
        o_sb = [actp.tile([128, S], FP16, name=f"o{m}") for m in range(2)]
        sums_sb = actp.tile([4, S], F32, name="sums")

        # ---------------- phase 1: QKV projections -------------------
        # kt-outer halves: 8 PSUM banks accumulate (t-half, m); matmuls
        # chase the per-kt x DMAs.
        with tc.tile_pool(name="pqk", bufs=1, space="PSUM") as pqkp:
            for half in range(2):
                ps = [[pqkp.tile([128, 512], F32, tag=f"pq{tt}_{m}",
                                 name=f"pq{tt}_{m}") for m in range(4)]
                      for tt in range(2)]
                if half == 0:
                    for w in range(24):
                        nc.tensor.matmul(ps[0][0][:, 0:128], perm_sb[:],
                                         perm_sb[:], start=True, stop=True)
                for kt in range(HT):
                    for tt in range(2):
                        tsl = bass.ts(half * 2 + tt, 512)
                        for m in range(4):
                            nc.tensor.matmul(
                                ps[tt][m][:], wqk_sb[:, kt, m * 128:(m + 1) * 128],
                                x_sb[:, kt, tsl],
                                start=(kt == 0), stop=(kt == HT - 1),
                            )
                for tt in range(2):
                    tsl = bass.ts(half * 2 + tt, 512)
                    for m in range(4):
                        if m % 2 == 0:
                            nc.scalar.activation(qkraw[m][:, tsl], ps[tt][m][:],
                                                 AF.Copy)
                        else:
                            nc.vector.tensor_copy(qkraw[m][:, tsl], ps[tt][m][:])

        if phases < 2:
            return
        # ---------------- phase 1b/2: v projection + RoPE -------------
        # Shared PSUM pool: v accumulators and RoPE rotation tiles.  The
        # rotation goes PSUM -> SBUF fp16 via Act so the DVE ops run
        # all-fp16 (2x rate) and the PSUM lifetime is short.
        with tc.tile_pool(name="pv", bufs=2, space="PSUM") as pvp, \
             tc.tile_pool(name="rtmp", bufs=4) as rtp:
            for t in range(KT):
                pv = pvp.tile([128, 260], F32, tag="pv", name="pv")
                for kt in range(HT):
                    nc.tensor.matmul(
                        pv[:], x_sb[:, kt, t * 128:(t + 1) * 128], wv_sb[:, kt, :],
                        start=(kt == 0), stop=(kt == HT - 1),
                    )
                if t % 2 == 0:
                    nc.scalar.activation(v_sb[t][:], pv[:], AF.Copy)
                else:
                    nc.vector.tensor_copy(v_sb[t][:], pv[:])
                nc.gpsimd.memset(v_sb[t][:, 64:260:65], 1.0)

            def _rope_one(m):
                dst = [qrope[0], qrope[1], krope[0], krope[1]][m]
                for t in range(QT):
                    tsl = bass.ts(t, 512)
                    sh = pvp.tile([128, 512], F32, tag="sh", name="sh")
                    nc.tensor.matmul(sh[:], perm_sb[:], qkraw[m][:, tsl],
                                     start=True, stop=True)
                    shs = rtp.tile([128, 512], FP16, tag="shs", name="shs")
                    nc.scalar.activation(shs[:], sh[:], AF.Copy)
                    t1 = rtp.tile([128, 512], FP16, tag="t1", name="t1")
                    nc.vector.tensor_mul(t1[:], qkraw[m][:, tsl], cos_sb[:, tsl])
                    t2 = rtp.tile([128, 512], FP16, tag="t2", name="t2")
                    nc.vector.tensor_mul(t2[:], shs[:], sin_sb[:, tsl])
                    nc.vector.tensor_add(dst[:, tsl], t1[:], t2[:])

            _rope_one(0); _rope_one(2)   # pair 0 q, k
            _rope_one(1); _rope_one(3)   # pair 1 q, k

        if phases < 3:
            return
        # -------- phase 3: attention + per-qb normalize/Wo ------------
        # PSUM (8 banks): st0,st1 [128,1024] (4) + 4x oP [128,512] (4).
        # Wo of qb reuses st tags (yp) and oP tags (bc) between blocks.
        rec_f = actp.tile([4, S], F32, name="rec_f")
        rec = actp.tile([4, S], F32R, name="rec")
        onorm = [actp.tile([128, S], FP16, name=f"onorm{m}") for m in range(2)]
        with tc.tile_pool(name="pst", bufs=1, space="PSUM") as pstp, \
             tc.tile_pool(name="pav", bufs=1, space="PSUM") as pavp, \
             tc.tile_pool(name="expp", bufs=2) as expp, \
             tc.tile_pool(name="schp", bufs=2) as schp, \
             tc.tile_pool(name="ysbp", bufs=4) as ysbp, \
             tc.tile_pool(name="ostg", bufs=2) as ostgp:

            def _block(p, qb):
                q0 = qb * 1024
                oP = [[pavp.tile([128, 512], F32, tag=f"o{hh}_{j}",
                                 name=f"o{hh}_{j}") for j in range(2)]
                      for hh in range(2)]
                sts = {}
                es = {}

                def _scores(kt):
                    for hh in range(2):
                        st = pstp.tile([128, 1024], F32, tag=f"st{hh}",
                                       name=f"st{hh}")
                        sts[(kt, hh)] = st
                        for j in range(2):
                            nc.tensor.matmul(
                                st[:, j * 512:(j + 1) * 512],
                                krope[p][hh * 64:hh * 64 + 64,
                                         kt * 128:(kt + 1) * 128],
                                qrope[p][hh * 64:hh * 64 + 64,
                                         q0 + j * 512:q0 + (j + 1) * 512],
                                start=True, stop=True)

                def _exp(kt):
                    for hh in range(2):
                        st = sts.pop((kt, hh))
                        e = expp.tile([128, 1024], FP16, tag=f"e{hh}",
                                      name=f"e{hh}")
                        es[(kt, hh)] = e
                        if hh == 1 and kt < DVE_KT:
                            ti = schp.tile([128, 1024], I32, tag="ti", name="ti")
                            nc.vector.tensor_scalar(
                                out=ti[:], in0=st[:], scalar1=A_SCH,
                                scalar2=schb_sb[:, kt:kt + 1],
                                op0=OP.mult, op1=OP.add)
                            nc.vector.tensor_scalar_max(
                                e[:], ti.bitcast(F32)[:], 0.0)
                        else:
                            nc.scalar.activation(
                                e[:], st[:], AF.Exp, bias=mask_sb[:, kt:kt + 1])

                def _av(kt):
                    for hh in range(2):
                        e = es.pop((kt, hh))
                        for j in range(2):
                            nc.tensor.matmul(
                                oP[hh][j][0:65, :],
                                v_sb[kt][:, (2 * p + hh) * 65:
                                         (2 * p + hh) * 65 + 65],
                                e[:, j * 512:(j + 1) * 512],
                                start=(kt == 0), stop=(kt == KT - 1))

                # software pipeline: scores(kt+1) issued before AV(kt)
                _scores(0)
                _exp(0)
                for kt in range(1, KT):
                    _scores(kt)
                    _av(kt - 1)
                    _exp(kt)
                _av(KT - 1)

                for hh in range(2):
                    h = 2 * p + hh
                    for j in range(2):
                        qsl = bass.ds(q0 + j * 512, 512)
                        stg = ostgp.tile([64, 512], FP16, tag=f"stg{hh}{j}",
                                         name=f"stg{hh}{j}")
                        nc.vector.tensor_copy(stg[:], oP[hh][j][0:64, :])
                        stg1 = ostgp.tile([1, 512], F32, tag=f"sg{hh}{j}",
                                          name=f"sg{hh}{j}")
                        nc.vector.tensor_copy(stg1[:], oP[hh][j][64:65, :])
                        nc.sync.dma_start(
                            out=o_sb[p][hh * 64:hh * 64 + 64, qsl],
                            in_=stg[:])
                        nc.sync.dma_start(out=sums_sb[h:h + 1, qsl],
                                          in_=stg1[:])
                # per-qb reciprocal once both pairs staged
                if p == 1:
                    qsl = bass.ds(q0, 1024)
                    nc.vector.reciprocal(rec_f[:, qsl], sums_sb[:, qsl])
                    nc.vector.tensor_copy(rec[:, qsl], rec_f[:, qsl])

            def _wo(qb):
                q0 = qb * 1024
                for m in range(2):
                    for t2 in range(2):
                        qsl = bass.ds(q0 + t2 * 512, 512)
                        bc = pavp.tile([128, 512], F32, tag=f"o{m}_{t2}",
                                       name=f"bc{m}{t2}")
                        if m == 0 and t2 == 0:
                            for w in range(8):
                                nc.tensor.matmul(bc[:, 0:128], perm_sb[:],
                                                 perm_sb[:], start=True,
                                                 stop=True)
                        nc.tensor.matmul(bc[:], sel_sb[:, m * 128:(m + 1) * 128],
                                         rec[:, qsl], start=True, stop=True)
                        nc.vector.tensor_mul(onorm[m][:, qsl], o_sb[m][:, qsl],
                                             bc[:])
                for mo in range(HT):
                    yp = pstp.tile([128, 1024], F32, tag=f"st{mo % 2}",
                                   name=f"yp{mo}")
                    for t2 in range(2):
                        qsl = bass.ds(q0 + t2 * 512, 512)
                        ysl = yp[:, t2 * 512:(t2 + 1) * 512]
                        for k2 in range(2):
                            nc.tensor.matmul(
                                ysl, wo_sb[:, k2, mo * 128:(mo + 1) * 128],
                                onorm[k2][:, qsl], start=(k2 == 0), stop=(k2 == 1))
                        ysb = ysbp.tile([128, 512], FP16, tag="ysb", name="ysb")
                        if (mo + t2) % 2 == 0:
                            nc.scalar.activation(ysb[:], ysl, AF.Copy)
                        else:
                            nc.vector.tensor_copy(ysb[:], ysl)
                        nc.sync.dma_start(
                            out=d["yT"][mo * 128:(mo + 1) * 128, qsl], in_=ysb[:])

            _block(0, 0)
            _block(1, 0)
            _block(0, 1)
            _wo(0)
            _block(1, 1)
            _wo(1)


def build(reps=1, split=True, phases=4):
    nc = bass.Bass("TRN2", target_bir_lowering=False, debug=False,
                   num_devices=N_CORES)
    d = {
        "xT": nc.dram_tensor("xT", [H, S], FP16, kind="ExternalInput"),
        "wqkT": nc.dram_tensor("wqkT", [H, 512], FP16, kind="ExternalInput"),
        "wvT": nc.dram_tensor("wvT", [H, 260], FP16, kind="ExternalInput"),
        "woT": nc.dram_tensor("woT", [DL, H], FP16, kind="ExternalInput"),
        "cosT": nc.dram_tensor("cosT", [128, S], FP16, kind="ExternalInput"),
        "sinT": nc.dram_tensor("sinT", [128, S], FP16, kind="ExternalInput"),
        "perm": nc.dram_tensor("perm", [128, 128], FP16, kind="ExternalInput"),
        "sel": nc.dram_tensor("sel", [4, 256], F32, kind="ExternalInput"),
        "maskb": nc.dram_tensor("maskb", [128, KT], F32, kind="ExternalInput"),
        "schb": nc.dram_tensor("schb", [128, KT], F32, kind="ExternalInput"),
        "yT": nc.dram_tensor("yT", [H, S], FP16, kind="ExternalOutput"),
    }
    with tile.TileContext(nc) as tc:
        if reps == 1:
            _emit_body(nc, tc, d, phases)
        else:
            with tc.For_i(0, reps, 1):
                _emit_body(nc, tc, d, phases)
    if split:
        _split_multiwait(nc)
    return nc


def host_inputs(x, attention_mask, Wq, Wk, Wv, Wo):
    """Build the 8 per-core input maps (numpy only)."""
    x = np.asarray(x, dtype=np.float32)
    attention_mask = np.asarray(attention_mask, dtype=np.float32)
    Wq = np.asarray(Wq, dtype=np.float32)
    Wk = np.asarray(Wk, dtype=np.float32)
    Wv = np.asarray(Wv, dtype=np.float32)
    Wo = np.asarray(Wo, dtype=np.float32)

    xT = [np.ascontiguousarray(x[b].T).astype(np.float16) for b in range(B)]

    p = np.arange(128)
    dd = p % HD
    inv = ROPE_BASE ** (-(dd % 32).astype(np.float32) / 32.0)
    s = np.arange(S, dtype=np.float32)
    ang = inv[:, None] * s[None, :]
    cosT = np.cos(ang).astype(np.float16)
    sinT = (np.where(dd < 32, -1.0, 1.0)[:, None] * np.sin(ang)).astype(np.float16)

    perm = np.zeros((128, 128), dtype=np.float16)
    for m in range(128):
        head, d_ = m // HD, m % HD
        perm[head * HD + (d_ + 32) % HD, m] = 1.0

    sel = np.zeros((4, 256), dtype=np.float32)
    for m in range(256):
        sel[m // HD, m] = 1.0

    maskb = [
        np.ascontiguousarray(
            (MASK_NEG * (1.0 - attention_mask[b])).astype(np.float32)
            .reshape(KT, 128).T
        )
        for b in range(B)
    ]
    schb = [(B_SCH + A_SCH * mb).astype(np.float32) for mb in maskb]

    in_maps = []
    for c in range(N_CORES):
        b, hq = c // 4, (c % 4) * HPC
        d0 = hq * HD
        wqkT = np.ascontiguousarray(
            np.concatenate([Wq[d0:d0 + DL] * (HD ** -0.5), Wk[d0:d0 + DL]], axis=0).T
        ).astype(np.float16)
        wvT = np.zeros((H, 260), dtype=np.float16)
        for hh in range(HPC):
            wvT[:, hh * 65:hh * 65 + 64] = Wv[d0 + hh * HD:d0 + (hh + 1) * HD].T
        woT = np.ascontiguousarray(Wo[:, d0:d0 + DL].T).astype(np.float16)
        in_maps.append({
            "xT": xT[b], "wqkT": wqkT, "wvT": wvT, "woT": woT,
            "cosT": cosT, "sinT": sinT, "perm": perm, "sel": sel,
            "maskb": maskb[b], "schb": schb[b],
        })
    return in_maps


def gather_output(results):
    y = np.zeros((B, S, H), dtype=np.float32)
    for c in range(N_CORES):
        y[c // 4] += results[c]["yT"].T.astype(np.float32)
    return y


_nc_cache = {}


def kernel(x, attention_mask, Wq, Wk, Wv, Wo):
    if "nc" not in _nc_cache:
        _nc_cache["nc"] = build(reps=1)
    nc = _nc_cache["nc"]
    in_maps = host_inputs(x, attention_mask, Wq, Wk, Wv, Wo)
    res = run_bass_kernel_spmd(nc, in_maps, list(range(N_CORES)), trace=False)
    return gather_output(res.results)
